# revision 27
# baseline (speedup 1.0000x reference)
"""Multi-head attention (strictly-upper-triangular mask variant) on 8 TRN2 cores.

Reference math (B=4, S=2048, D=512, H=8, A=64):
    q/k/v = per-head projections of query/key/value           [B,H,S,A]
    scores = q @ k^T / sqrt(A), masked where k <= q (lower triangle incl diag
    masked to -1e9 -> softmax attends strictly to FUTURE positions)
    out = concat_heads(softmax(scores) @ v) @ Wo + bo         [B,S,D]

Sharding: 8 cores = 4 batches x 2 interleaved q-tile sets.  Core c handles
batch b=c//2, q-tiles g = 2*i + (c%2) for i in 0..7 (128 rows each).  Every
core computes all 8 heads for its 1024 query rows; no collectives needed -
the host gather is a pure row-interleave concat.

v2 changes over the first working kernel (214us):
  * chunked input DMA ordered by first use, so the PE starts within ~2us
    instead of waiting ~24us for whole-tensor loads
  * V is projected directly into natural [k, a] layout (stationary = x^T
    key-tile, moving = all 8 heads' Wv) - eliminates 64 PE transposes and
    their PSUM/DVE traffic; the per-head [V | ones] stationary blocks for
    the AV matmul are assembled by one strided DVE copy per key tile
  * even/odd heads of a pair are interleaved per score matmul: the K=64
    stationaries auto-place on PE row-groups (0,0)/(64,0), so consecutive
    matmuls run concurrently in the array and their LDWEIGHTS overlap
  * score strips bin-packed into six [128,1536] fp32 PSUM groups per head
    (exact fit), exp'd in 6 ACT calls; PSUM budget 2x3 banks scores +
    2x1 bank shared (AV / projections / output) = 8
  * bv is folded into the output-projection bias on the host
    (attn@(V+1*bv^T) = attn@V + denom*bv -> normalized +bv -> bo' =
    bo + concat(bv)@Wo), so the V eviction is a plain copy
  * deep cross-pair pipeline: AV of pair p and projections of pair p+1
    fill the PE while pair p+1's exps run on ACT, keeping the PE HAM
    clock-gate warm

The single fully-masked query row (q = S-1, uniform attention in the
reference) comes back NaN from the device and is recomputed exactly on the
host during the gather.
"""

import numpy as np
import ml_dtypes

B, S, D, H, A = 4, 2048, 512, 8, 64
P = 128
NQ = 1024          # q rows per core
NQT = 8            # q tiles per core
NKC = 16           # k chunks
NPAIR = 4          # head pairs
BF = ml_dtypes.bfloat16

WKC = [P * (kc // 2 + 1) for kc in range(NKC)]   # score-strip width per k chunk
GW = 1536                                        # score psum group width (3 banks)

# strips bin-packed into six [128,1536] groups per head: SEG[kc] = (group, off)
SEG = {
    14: (0, 0), 6: (0, 1024),
    15: (1, 0), 7: (1, 1024),
    12: (2, 0), 4: (2, 896), 2: (2, 1280),
    13: (3, 0), 5: (3, 896), 3: (3, 1280),
    8: (4, 0), 9: (4, 640), 0: (4, 1280), 1: (4, 1408),
    10: (5, 0), 11: (5, 768),
}
GRP_KCS = [[] for _ in range(6)]
for _kc in range(NKC):
    GRP_KCS[SEG[_kc][0]].append(_kc)

_cache = {}


def _split512(a, b):
    """Split [a,b) at multiples of 512 (PSUM bank boundaries)."""
    out = []
    while a < b:
        nxt = min(b, (a // 512 + 1) * 512)
        out.append((a, nxt))
        a = nxt
    return out


def _build():
    if "nc" in _cache:
        return _cache["nc"]

    import concourse.bacc as bacc
    import concourse.mybir as mybir
    import concourse.tile as tile

    F32 = mybir.dt.float32
    BF16 = mybir.dt.bfloat16
    MULT = mybir.AluOpType.mult
    EXP = mybir.ActivationFunctionType.Exp

    nc = bacc.Bacc("TRN2", target_bir_lowering=False, debug=False, num_devices=8)

    # big inputs are chunk-major [n, P, w] so every DMA reads DRAM sequentially
    qT8_d = nc.dram_tensor("qT8", [8, P, 512], BF16, kind="ExternalInput")
    kT_d = nc.dram_tensor("kT", [16, P, 512], BF16, kind="ExternalInput")
    vTk_d = nc.dram_tensor("vTk", [16, P, 512], BF16, kind="ExternalInput")
    wq_d = nc.dram_tensor("wq", [4, P, 512], BF16, kind="ExternalInput")
    wk_d = nc.dram_tensor("wk", [4, P, 512], BF16, kind="ExternalInput")
    wv_d = nc.dram_tensor("wvN", [4, P, 512], BF16, kind="ExternalInput")
    wo_d = nc.dram_tensor("wo", [4, P, 512], BF16, kind="ExternalInput")
    bq_d = nc.dram_tensor("bq8", [P, 4], F32, kind="ExternalInput")
    bk_d = nc.dram_tensor("bk", [P, 4], F32, kind="ExternalInput")
    bo_d = nc.dram_tensor("boB", [P, D], BF16, kind="ExternalInput")
    me_d = nc.dram_tensor("maskE", [P, P], BF16, kind="ExternalInput")
    mo_d = nc.dram_tensor("maskO", [P, P], BF16, kind="ExternalInput")
    out_d = nc.dram_tensor("out", [NQ, D], F32, kind="ExternalOutput")

    with tile.TileContext(nc) as tc:
        with (
            tc.tile_pool(name="cst", bufs=1) as cst,
            tc.tile_pool(name="act", bufs=1) as act,
            tc.tile_pool(name="ptg", bufs=16) as ptg,
            tc.tile_pool(name="rcp", bufs=3) as rcp,
            tc.tile_pool(name="ost", bufs=2) as ost,
            tc.tile_pool(name="sc", bufs=2, space="PSUM") as sc,
            tc.tile_pool(name="sm", bufs=2, space="PSUM") as sm,
        ):
            qT8 = cst.tile([P, 4 * NQ], BF16, tag="qT8")
            kT = cst.tile([P, 4 * S], BF16, tag="kT")
            vTk = cst.tile([P, NKC * 512], BF16, tag="vTk")
            wq = cst.tile([P, 2048], BF16, tag="wq")
            wk = cst.tile([P, 2048], BF16, tag="wk")
            wv = cst.tile([P, 2048], BF16, tag="wv")
            wo = cst.tile([P, 2048], BF16, tag="wo")
            bq = cst.tile([P, 4], F32, tag="bq")
            bk = cst.tile([P, 4], F32, tag="bk")
            boB = cst.tile([P, D], BF16, tag="boB")
            mE = cst.tile([P, P], BF16, tag="mE")
            mO = cst.tile([P, P], BF16, tag="mO")

            # small tensors first, then big ones chunked in first-use order
            for t, d in [(bq, bq_d), (bk, bk_d), (boB, bo_d),
                         (mE, me_d), (mO, mo_d)]:
                nc.sync.dma_start(t[:], d[:])

            def dma_chunks(t, d, order):
                for c in order:
                    nc.sync.dma_start(t[:, 512 * c:512 * (c + 1)], d[c, :, :])

            # chunk order = first-use order of the projection block loops
            dma_chunks(wq, wq_d, range(4))
            dma_chunks(qT8, qT8_d, [0, 2, 4, 6, 1, 3, 5, 7])
            dma_chunks(wk, wk_d, range(4))
            dma_chunks(kT, kT_d, [4 * c + s for s in (2, 0, 1, 3) for c in range(4)])
            dma_chunks(wv, wv_d, range(4))
            dma_chunks(vTk, vTk_d, range(16))
            dma_chunks(wo, wo_d, range(4))

            QT = [act.tile([P, NQ], BF16, tag=f"QT{p}", name=f"QT{p}") for p in range(NPAIR)]
            KT = [act.tile([P, S], BF16, tag=f"KT{p}", name=f"KT{p}") for p in range(NPAIR)]
            XT = [act.tile([P, NQ], BF16, tag=f"XT{c}", name=f"XT{c}") for c in range(4)]
            # per k-chunk: [V_h0 | ones | V_h1 | ones | ... | V_h7 | ones]
            Vk = act.tile([P, NKC * 1024], BF16, tag="Vk", name="Vk")

            nc.gpsimd.memset(
                Vk[:].rearrange("p (x t c) -> p x t c", t=2, c=64)[:, :, 1, :], 1.0)

            # ---- V directly in natural [k, a] layout for all 8 heads ----
            def vdirect():
                for kt in range(NKC):
                    psV = sm.tile([P, 512], F32, tag="sm")
                    for ch in range(4):
                        nc.tensor.matmul(
                            psV[:], vTk[:, 512 * kt + P * ch: 512 * kt + P * (ch + 1)],
                            wv[:, 512 * ch:512 * (ch + 1)],
                            start=(ch == 0), stop=(ch == 3))
                    dst = Vk[:].rearrange("p (k h t c) -> p k h t c", k=NKC, h=8, c=64)
                    nc.vector.tensor_copy(
                        dst[:, kt, :, 0, :],
                        psV[:].rearrange("p (h c) -> p h c", c=64))

            # ---- Q/K projections for one head pair ----
            def projQK(p):
                for qh in range(NQ // 512):
                    ps = sm.tile([P, 512], F32, tag="sm")
                    for ch in range(4):
                        nc.tensor.matmul(
                            ps[:], wq[:, (4 * p + ch) * P:(4 * p + ch + 1) * P],
                            qT8[:, NQ * ch + 512 * qh: NQ * ch + 512 * (qh + 1)],
                            start=(ch == 0), stop=(ch == 3))
                    nc.vector.tensor_scalar_add(
                        QT[p][:, 512 * qh:512 * (qh + 1)], ps[:], bq[:, p:p + 1])
                # sh order matches score-group first-use: groups 4,5 (k
                # chunks 8..11) only need K block sh=2 plus sh=0/1
                for sh in (2, 0, 1, 3):
                    ps = sm.tile([P, 512], F32, tag="sm")
                    for ch in range(4):
                        nc.tensor.matmul(
                            ps[:], wk[:, (4 * p + ch) * P:(4 * p + ch + 1) * P],
                            kT[:, S * ch + 512 * sh: S * ch + 512 * (sh + 1)],
                            start=(ch == 0), stop=(ch == 3))
                    nc.vector.tensor_scalar_add(
                        KT[p][:, 512 * sh:512 * (sh + 1)], ps[:], bk[:, p:p + 1])

            # ---- scores + exp + mask for one head pair, heads interleaved ----
            def scores_pair(p):
                pts = [[None] * 6, [None] * 6]     # [hh][group]
                # groups 4,5 first: their K chunks project earliest
                for g in (4, 5, 2, 3, 0, 1):
                    sE = sc.tile([P, GW], F32, tag="sc")
                    sO = sc.tile([P, GW], F32, tag="sc")
                    for kc in GRP_KCS[g]:
                        off = SEG[kc][1]
                        for (a0, a1) in _split512(off, off + WKC[kc]):
                            nc.tensor.matmul(
                                sE[:, a0:a1], KT[p][0:64, P * kc:P * (kc + 1)],
                                QT[p][0:64, a0 - off:a1 - off],
                                start=True, stop=True)
                            nc.tensor.matmul(
                                sO[:, a0:a1], KT[p][64:128, P * kc:P * (kc + 1)],
                                QT[p][64:128, a0 - off:a1 - off],
                                start=True, stop=True)
                    for hh, sT in ((0, sE), (1, sO)):
                        pt = ptg.tile([P, GW], BF16, tag="pt")
                        nc.scalar.activation(pt[:], sT[:], EXP)
                        for kc in GRP_KCS[g]:
                            off = SEG[kc][1]
                            eng = nc.vector if kc % 4 == 0 else nc.gpsimd
                            eng.tensor_tensor(
                                pt[:, off + WKC[kc] - P: off + WKC[kc]],
                                pt[:, off + WKC[kc] - P: off + WKC[kc]],
                                mE[:] if kc % 2 == 0 else mO[:], MULT)
                        pts[hh][g] = pt
                return pts

            # AV accumulation per head: one psum bank per 512 q columns;
            # start=True only on the bank's first matmul (clears has_written
            # for the whole bank), later start=False matmuls
            # overwrite-where-unset / accumulate-where-set.
            # AV accumulation per head: one psum bank per 512 q columns;
            # start=True only on the bank's first matmul (clears has_written
            # for the whole bank), later start=False matmuls
            # overwrite-where-unset / accumulate-where-set.  Issued kc-major
            # AFTER the pair's exps so each bank is held only briefly.
            def av_pair(p, pts):
                for b in range(2):
                    for hh in (0, 1):
                        h = 2 * p + hh
                        hr = slice(64 * hh, 64 * hh + 64)
                        avb = sm.tile([P, 512], F32, tag="sm")
                        kc0 = 8 * b
                        for kc in range(kc0, NKC):
                            g, off = SEG[kc]
                            w = min(WKC[kc], 512 * (b + 1)) - 512 * b
                            nc.tensor.matmul(
                                avb[:, 0:w],
                                Vk[:, 1024 * kc + P * h: 1024 * kc + P * (h + 1)],
                                pts[hh][g][:, off + 512 * b: off + 512 * b + w],
                                start=(kc == kc0), stop=(kc == 15),
                                skip_group_check=True)
                        # reciprocal is a custom DVE op that cannot read PSUM;
                        # bounce the replicated denominators through SBUF
                        r = rcp.tile([64, 1024], F32, tag="rec")
                        nc.vector.tensor_copy(r[:, 0:512], avb[64:128, :])
                        nc.vector.reciprocal_approx_fast(r[:, 512:1024], r[:, 0:512])
                        nc.vector.tensor_tensor(
                            XT[p][hr, 512 * b:512 * (b + 1)],
                            avb[0:64, :], r[:, 512:1024], MULT)

            # issue order = scheduler priority.  Per steady-state pair p:
            # scores(p) outrank the PE fillers (previous pair's AV, next
            # pair's projections) so the ACT exp stream never starves, and
            # the fillers outrank later pairs' psum-slot requests so slot
            # reservations don't serialize the pipeline.
            projQK(0)
            for p in range(NPAIR):
                pts = scores_pair(p)
                if p == 0:
                    vdirect()
                if p + 1 < NPAIR:
                    projQK(p + 1)
                av_pair(p, pts)

            # ---- output projection ----
            for i in range(NQT):
                po = sm.tile([P, D], F32, tag="sm")
                for ch in range(4):
                    nc.tensor.matmul(po[:], XT[ch][:, P * i:P * (i + 1)],
                                     wo[:, 512 * ch:512 * (ch + 1)],
                                     start=(ch == 0), stop=(ch == 3))
                ob = ost.tile([P, D], F32, tag="ob")
                nc.vector.tensor_tensor(ob[:], po[:], boB[:],
                                        mybir.AluOpType.add)
                nc.sync.dma_start(out_d[P * i:P * (i + 1), :], ob[:])

    nc.compile()
    _cache["nc"] = nc
    return nc


def _host_prep(query, key, value, Wq, bq, Wk, bk, Wv, bv, Wo, bo):
    """Build the 8 per-core input maps (all device-side layouts)."""
    def stack_pairs(W):
        # [H,D,A] -> [128, 16*128]: col block (4p+ch) = rows 128ch of [W_2p|W_2p+1]
        blocks = []
        for p in range(NPAIR):
            Wp = np.concatenate([W[2 * p], W[2 * p + 1]], axis=1)  # [512, 128]
            for ch in range(4):
                blocks.append(Wp[P * ch:P * (ch + 1), :])
        return np.stack(blocks, 1).reshape(P, -1).astype(BF)

    wq_h, wk_h = stack_pairs(Wq), stack_pairs(Wk)
    # all-heads Wv, chunked by feature rows: col block ch = WvCat[128ch:128ch+128]
    WvCat = np.concatenate([Wv[h] for h in range(H)], axis=1)      # [512, 512]
    wv_h = WvCat.reshape(4, P, 512).transpose(1, 0, 2).reshape(P, -1).astype(BF)
    wo_h = np.stack([Wo[P * ch:P * (ch + 1), :] for ch in range(4)], 1)
    wo_h = wo_h.reshape(P, -1).astype(BF)

    def stack_bias(b, scale=1.0):
        cols = [np.concatenate([b[2 * p], b[2 * p + 1]]) * scale for p in range(NPAIR)]
        return np.stack(cols, 1).astype(np.float32)

    bq_h = stack_bias(bq, 0.125)
    bk_h = stack_bias(bk)
    # bv folded into the output bias: attn@(V + 1 bv^T) normalizes to +bv
    boP = (bo + np.concatenate([bv[h] for h in range(H)]) @ Wo).astype(BF)
    boB = np.ascontiguousarray(np.broadcast_to(boP, (P, D)))
    kl = np.arange(P)[:, None]
    ql = np.arange(P)[None, :]
    tril_strict = (kl > ql).astype(BF)

    def dram_chunks(m):
        # [128, n*512] SBUF image -> chunk-major [n, 128, 512] DRAM layout
        n = m.shape[1] // 512
        return np.ascontiguousarray(m.reshape(P, n, 512).transpose(1, 0, 2))

    def chunked_T(x, scale=1.0):
        # [S', D] -> [128, 4*S'] with col block ch = rows 128ch of x.T
        xT = np.ascontiguousarray(x.T) * scale
        return xT.reshape(4, P, -1).transpose(1, 0, 2).reshape(P, -1).astype(BF)

    def kmajor_T(x):
        # [S, D] -> [128, 16*512]: col 512*kt + 128*ch + c = x[128*kt+c, 128*ch+r]
        v4 = x.reshape(NKC, P, 4, P)            # (kt, c, ch, r)
        return np.ascontiguousarray(
            v4.transpose(3, 0, 2, 1)).reshape(P, -1).astype(BF)

    wq_h, wk_h, wv_h, wo_h = map(dram_chunks, (wq_h, wk_h, wv_h, wo_h))

    in_maps = []
    for c in range(8):
        b, pair = c // 2, c % 2
        sel = np.concatenate(
            [np.arange(P * (2 * i + pair), P * (2 * i + pair) + P) for i in range(NQT)])
        m = {
            "qT8": dram_chunks(chunked_T(query[b][sel], 0.125)),
            "kT": dram_chunks(chunked_T(key[b])),
            "vTk": dram_chunks(kmajor_T(value[b])),
            "wq": wq_h, "wk": wk_h, "wvN": wv_h, "wo": wo_h,
            "bq8": bq_h, "bk": bk_h, "boB": boB,
            "maskE": tril_strict if pair == 0 else np.zeros((P, P), BF),
            "maskO": np.ones((P, P), BF) if pair == 0 else tril_strict,
        }
        in_maps.append(m)
    return in_maps


def kernel(query, key, value, Wq, bq, Wk, bk, Wv, bv, Wo, bo):
    from concourse.bass_utils import run_bass_kernel_spmd

    args = [np.asarray(a, dtype=np.float32) for a in
            (query, key, value, Wq, bq, Wk, bk, Wv, bv, Wo, bo)]
    query, key, value, Wq, bq, Wk, bk, Wv, bv, Wo, bo = args

    nc = _build()
    in_maps = _host_prep(*args)
    res = run_bass_kernel_spmd(nc, in_maps, list(range(8)))

    out = np.empty((B, S, D), np.float32)
    for c in range(8):
        b, pair = c // 2, c % 2
        o = res.results[c]["out"]
        for i in range(NQT):
            g = 2 * i + pair
            out[b, P * g:P * (g + 1), :] = o[P * i:P * (i + 1), :]

    # q = S-1 attends to nothing -> reference softmax is uniform over all keys
    for b in range(B):
        vm = value[b].mean(0)
        x = np.concatenate([vm @ Wv[h] + bv[h] for h in range(H)])
        out[b, S - 1, :] = x @ Wo + bo
    return out


# revision 31
# speedup vs baseline: 1.1160x; 1.1160x over previous
"""Multi-head attention (strictly-upper-triangular mask variant) on 8 TRN2 cores.

Reference math (B=4, S=2048, D=512, H=8, A=64):
    q/k/v = per-head projections of query/key/value           [B,H,S,A]
    scores = q @ k^T / sqrt(A), masked where k <= q (lower triangle incl diag
    masked to -1e9 -> softmax attends strictly to FUTURE positions)
    out = concat_heads(softmax(scores) @ v) @ Wo + bo         [B,S,D]

Sharding: 8 cores = 4 batches x 2 interleaved q-tile sets.  Core c handles
batch b=c//2, q-tiles g = 2*i + (c%2) for i in 0..7 (128 rows each).  Every
core computes all 8 heads for its 1024 query rows; no collectives needed -
the host gather is a pure row-interleave concat.

v2 changes over the first working kernel (214us):
  * chunked input DMA ordered by first use, so the PE starts within ~2us
    instead of waiting ~24us for whole-tensor loads
  * V is projected directly into natural [k, a] layout (stationary = x^T
    key-tile, moving = all 8 heads' Wv) - eliminates 64 PE transposes and
    their PSUM/DVE traffic; the per-head [V | ones] stationary blocks for
    the AV matmul are assembled by one strided DVE copy per key tile
  * even/odd heads of a pair are interleaved per score matmul: the K=64
    stationaries auto-place on PE row-groups (0,0)/(64,0), so consecutive
    matmuls run concurrently in the array and their LDWEIGHTS overlap
  * score strips bin-packed into six [128,1536] fp32 PSUM groups per head
    (exact fit), exp'd in 6 ACT calls; PSUM budget 2x3 banks scores +
    2x1 bank shared (AV / projections / output) = 8
  * bv is folded into the output-projection bias on the host
    (attn@(V+1*bv^T) = attn@V + denom*bv -> normalized +bv -> bo' =
    bo + concat(bv)@Wo), so the V eviction is a plain copy
  * deep cross-pair pipeline: AV of pair p and projections of pair p+1
    fill the PE while pair p+1's exps run on ACT, keeping the PE HAM
    clock-gate warm

The single fully-masked query row (q = S-1, uniform attention in the
reference) comes back NaN from the device and is recomputed exactly on the
host during the gather.
"""

import numpy as np
import ml_dtypes

B, S, D, H, A = 4, 2048, 512, 8, 64
P = 128
NQ = 1024          # q rows per core
NQT = 8            # q tiles per core
NKC = 16           # k chunks
NPAIR = 4          # head pairs
BF = ml_dtypes.bfloat16

WKC = [P * (kc // 2 + 1) for kc in range(NKC)]   # score-strip width per k chunk
COFF = [0]
for _w in WKC:
    COFF.append(COFF[-1] + _w)                   # kc-ordered strip offsets, total 9216
WIN = 768                                        # score window width per head
NWIN = COFF[-1] // WIN                           # 12 windows
GW = 2 * WIN                                     # psum tile: [E window | O window]


def _pieces(c0, c1, splits):
    """Cut global column range [c0,c1) at window boundaries and at the
    window-local offsets in `splits`; yield (window, l0, l1)."""
    a = c0
    while a < c1:
        w = a // WIN
        nxt = WIN * (w + 1)
        for s in splits:
            g = WIN * w + s
            if a < g < nxt:
                nxt = g
        b = min(c1, nxt)
        yield (w, a - WIN * w, b - WIN * w)
        a = b

_cache = {}


def _split512(a, b):
    """Split [a,b) at multiples of 512 (PSUM bank boundaries)."""
    out = []
    while a < b:
        nxt = min(b, (a // 512 + 1) * 512)
        out.append((a, nxt))
        a = nxt
    return out


def _build():
    if "nc" in _cache:
        return _cache["nc"]

    import concourse.bacc as bacc
    import concourse.mybir as mybir
    import concourse.tile as tile

    F32 = mybir.dt.float32
    BF16 = mybir.dt.bfloat16
    MULT = mybir.AluOpType.mult
    EXP = mybir.ActivationFunctionType.Exp

    nc = bacc.Bacc("TRN2", target_bir_lowering=False, debug=False, num_devices=8)

    # big inputs are chunk-major [n, P, w] so every DMA reads DRAM sequentially
    qT8_d = nc.dram_tensor("qT8", [8, P, 512], BF16, kind="ExternalInput")
    kT_d = nc.dram_tensor("kT", [16, P, 512], BF16, kind="ExternalInput")
    vTk_d = nc.dram_tensor("vTk", [16, P, 512], BF16, kind="ExternalInput")
    wq_d = nc.dram_tensor("wq", [4, P, 512], BF16, kind="ExternalInput")
    wk_d = nc.dram_tensor("wk", [4, P, 512], BF16, kind="ExternalInput")
    wv_d = nc.dram_tensor("wvN", [4, P, 512], BF16, kind="ExternalInput")
    wo_d = nc.dram_tensor("wo", [4, P, 512], BF16, kind="ExternalInput")
    bq_d = nc.dram_tensor("bq8", [P, 4], F32, kind="ExternalInput")
    bk_d = nc.dram_tensor("bk", [P, 4], F32, kind="ExternalInput")
    bo_d = nc.dram_tensor("boB", [P, D], BF16, kind="ExternalInput")
    me_d = nc.dram_tensor("maskE", [P, P], BF16, kind="ExternalInput")
    mo_d = nc.dram_tensor("maskO", [P, P], BF16, kind="ExternalInput")
    out_d = nc.dram_tensor("out", [NQ, D], F32, kind="ExternalOutput")

    with tile.TileContext(nc) as tc:
        with (
            tc.tile_pool(name="cst", bufs=1) as cst,
            tc.tile_pool(name="act", bufs=1) as act,
            tc.tile_pool(name="ptg", bufs=16) as ptg,
            tc.tile_pool(name="rcp", bufs=3) as rcp,
            tc.tile_pool(name="ost", bufs=2) as ost,
            tc.tile_pool(name="sc", bufs=2, space="PSUM") as sc,
            tc.tile_pool(name="sm", bufs=2, space="PSUM") as sm,
        ):
            qT8 = cst.tile([P, 4 * NQ], BF16, tag="qT8")
            kT = cst.tile([P, 4 * S], BF16, tag="kT")
            vTk = cst.tile([P, NKC * 512], BF16, tag="vTk")
            wq = cst.tile([P, 2048], BF16, tag="wq")
            wk = cst.tile([P, 2048], BF16, tag="wk")
            wv = cst.tile([P, 2048], BF16, tag="wv")
            wo = cst.tile([P, 2048], BF16, tag="wo")
            bq = cst.tile([P, 4], F32, tag="bq")
            bk = cst.tile([P, 4], F32, tag="bk")
            boB = cst.tile([P, D], BF16, tag="boB")
            mE = cst.tile([P, P], BF16, tag="mE")
            mO = cst.tile([P, P], BF16, tag="mO")

            # small tensors first, then big ones chunked in first-use order
            for t, d in [(bq, bq_d), (bk, bk_d), (boB, bo_d),
                         (mE, me_d), (mO, mo_d)]:
                nc.sync.dma_start(t[:], d[:])

            def dma_chunks(t, d, order):
                for c in order:
                    nc.sync.dma_start(t[:, 512 * c:512 * (c + 1)], d[c, :, :])

            # chunk order = first-use order of the projection block loops
            dma_chunks(wq, wq_d, range(4))
            dma_chunks(qT8, qT8_d, [0, 2, 4, 6, 1, 3, 5, 7])
            dma_chunks(wk, wk_d, range(4))
            dma_chunks(kT, kT_d, [4 * c + s for s in range(4) for c in range(4)])
            dma_chunks(wv, wv_d, range(4))
            dma_chunks(vTk, vTk_d, range(16))
            dma_chunks(wo, wo_d, range(4))

            QT = [act.tile([P, NQ], BF16, tag=f"QT{p}", name=f"QT{p}") for p in range(NPAIR)]
            KT = [act.tile([P, S], BF16, tag=f"KT{p}", name=f"KT{p}") for p in range(NPAIR)]
            XT = [act.tile([P, NQ], BF16, tag=f"XT{c}", name=f"XT{c}") for c in range(4)]
            # per k-chunk: [V_h0 | ones | V_h1 | ones | ... | V_h7 | ones]
            Vk = act.tile([P, NKC * 1024], BF16, tag="Vk", name="Vk")

            nc.gpsimd.memset(
                Vk[:].rearrange("p (x t c) -> p x t c", t=2, c=64)[:, :, 1, :], 1.0)

            # ---- V directly in natural [k, a] layout for all 8 heads ----
            def vdirect():
                for kt in range(NKC):
                    psV = sm.tile([P, 512], F32, tag="sm")
                    for ch in range(4):
                        nc.tensor.matmul(
                            psV[:], vTk[:, 512 * kt + P * ch: 512 * kt + P * (ch + 1)],
                            wv[:, 512 * ch:512 * (ch + 1)],
                            start=(ch == 0), stop=(ch == 3))
                    dst = Vk[:].rearrange("p (k h t c) -> p k h t c", k=NKC, h=8, c=64)
                    nc.vector.tensor_copy(
                        dst[:, kt, :, 0, :],
                        psV[:].rearrange("p (h c) -> p h c", c=64))

            # ---- Q/K projections for one head pair ----
            def projQK(p):
                for qh in range(NQ // 512):
                    ps = sm.tile([P, 512], F32, tag="sm")
                    for ch in range(4):
                        nc.tensor.matmul(
                            ps[:], wq[:, (4 * p + ch) * P:(4 * p + ch + 1) * P],
                            qT8[:, NQ * ch + 512 * qh: NQ * ch + 512 * (qh + 1)],
                            start=(ch == 0), stop=(ch == 3))
                    nc.vector.tensor_scalar_add(
                        QT[p][:, 512 * qh:512 * (qh + 1)], ps[:], bq[:, p:p + 1])
                for sh in range(S // 512):
                    ps = sm.tile([P, 512], F32, tag="sm")
                    for ch in range(4):
                        nc.tensor.matmul(
                            ps[:], wk[:, (4 * p + ch) * P:(4 * p + ch + 1) * P],
                            kT[:, S * ch + 512 * sh: S * ch + 512 * (sh + 1)],
                            start=(ch == 0), stop=(ch == 3))
                    nc.vector.tensor_scalar_add(
                        KT[p][:, 512 * sh:512 * (sh + 1)], ps[:], bk[:, p:p + 1])

            # ---- scores + exp + mask for one head pair ----
            # One [128, 2*WIN] psum tile per 768-wide score window holds BOTH
            # heads (E cols 0:768, O cols 768:1536): the even/odd matmuls of a
            # strip become co-ready and run concurrently on PE row-groups
            # (0,0)/(64,0), and one exp covers both heads.  Matmul pieces are
            # cut at window boundaries and at psum bank boundaries (tile-local
            # 512 for E, 256 for O).
            def scores_pair(p):
                ptw = []
                for w in range(NWIN):
                    sw = sc.tile([P, GW], F32, tag="sc")
                    for kc in range(NKC):
                        c0, c1 = COFF[kc], COFF[kc + 1]
                        w0, w1 = max(c0, WIN * w), min(c1, WIN * (w + 1))
                        if w0 >= w1:
                            continue
                        pcs = [(0, list(_pieces(w0, w1, (512,)))),
                               (1, list(_pieces(w0, w1, (256,))))]
                        for i in range(max(len(pcs[0][1]), len(pcs[1][1]))):
                            for hh, pl in pcs:
                                if i >= len(pl):
                                    continue
                                _, l0, l1 = pl[i]
                                nc.tensor.matmul(
                                    sw[:, WIN * hh + l0: WIN * hh + l1],
                                    KT[p][64 * hh:64 * hh + 64, P * kc:P * (kc + 1)],
                                    QT[p][64 * hh:64 * hh + 64,
                                          WIN * w + l0 - c0: WIN * w + l1 - c0],
                                    start=True, stop=True)
                    pt = ptg.tile([P, GW], BF16, tag="pt")
                    nc.scalar.activation(pt[:], sw[:], EXP)
                    # diagonal masks whose last 128 columns land in this window
                    for kc in range(NKC):
                        d0 = COFF[kc + 1] - P
                        if not (WIN * w <= d0 < WIN * (w + 1)):
                            continue
                        ld = d0 - WIN * w
                        m = mE[:] if kc % 2 == 0 else mO[:]
                        for hh in (0, 1):
                            eng = nc.vector if (kc + hh) % 2 == 0 else nc.gpsimd
                            eng.tensor_tensor(
                                pt[:, WIN * hh + ld: WIN * hh + ld + P],
                                pt[:, WIN * hh + ld: WIN * hh + ld + P],
                                m, MULT)
                    ptw.append(pt)
                return ptw

            # AV accumulation per head: one psum bank per 512 q columns;
            # start=True only on the bank's first matmul (clears has_written
            # for the whole bank), later start=False matmuls
            # overwrite-where-unset / accumulate-where-set.  Issued kc-major
            # AFTER the pair's exps so each bank is held only briefly; pieces
            # are cut at score-window boundaries.
            def av_pair(p, ptw):
                for hh in (0, 1):
                    h = 2 * p + hh
                    hr = slice(64 * hh, 64 * hh + 64)
                    for b in range(2):
                        avb = sm.tile([P, 512], F32, tag="sm")
                        # kc DESCENDING: the first matmul (and hence the psum
                        # slot request) depends on the LAST score window, so
                        # the bank is claimed only once the pair's exps are
                        # done and is held briefly
                        work = []
                        for kc in reversed(range(8 * b, NKC)):
                            c0 = COFF[kc] + 512 * b
                            c1 = COFF[kc] + min(WKC[kc], 512 * (b + 1))
                            for (w, l0, l1) in _pieces(c0, c1, ()):
                                work.append((kc, w, l0, l1, c0))
                        for i, (kc, w, l0, l1, c0) in enumerate(work):
                            o0 = WIN * w + l0 - c0
                            nc.tensor.matmul(
                                avb[:, o0:o0 + (l1 - l0)],
                                Vk[:, 1024 * kc + P * h: 1024 * kc + P * (h + 1)],
                                ptw[w][:, WIN * hh + l0: WIN * hh + l1],
                                start=(i == 0), stop=(i == len(work) - 1),
                                skip_group_check=True)
                        # reciprocal is a custom DVE op that cannot read PSUM;
                        # bounce the replicated denominators through SBUF
                        r = rcp.tile([64, 1024], F32, tag="rec")
                        nc.vector.tensor_copy(r[:, 0:512], avb[64:128, :])
                        nc.vector.reciprocal_approx_fast(r[:, 512:1024], r[:, 0:512])
                        nc.vector.tensor_tensor(
                            XT[p][hr, 512 * b:512 * (b + 1)],
                            avb[0:64, :], r[:, 512:1024], MULT)

            # issue order = scheduler priority.  Per steady-state pair p:
            # scores(p) outrank the PE fillers (previous pair's AV, next
            # pair's projections) so the ACT exp stream never starves, and
            # the fillers outrank later pairs' psum-slot requests so slot
            # reservations don't serialize the pipeline.
            projQK(0)
            for p in range(NPAIR):
                pts = scores_pair(p)
                if p == 0:
                    vdirect()
                if p + 1 < NPAIR:
                    projQK(p + 1)
                av_pair(p, pts)

            # ---- output projection ----
            for i in range(NQT):
                po = sm.tile([P, D], F32, tag="sm")
                for ch in range(4):
                    nc.tensor.matmul(po[:], XT[ch][:, P * i:P * (i + 1)],
                                     wo[:, 512 * ch:512 * (ch + 1)],
                                     start=(ch == 0), stop=(ch == 3))
                ob = ost.tile([P, D], F32, tag="ob")
                nc.vector.tensor_tensor(ob[:], po[:], boB[:],
                                        mybir.AluOpType.add)
                nc.sync.dma_start(out_d[P * i:P * (i + 1), :], ob[:])

    nc.compile()
    _cache["nc"] = nc
    return nc


def _host_prep(query, key, value, Wq, bq, Wk, bk, Wv, bv, Wo, bo):
    """Build the 8 per-core input maps (all device-side layouts)."""
    def stack_pairs(W):
        # [H,D,A] -> [128, 16*128]: col block (4p+ch) = rows 128ch of [W_2p|W_2p+1]
        blocks = []
        for p in range(NPAIR):
            Wp = np.concatenate([W[2 * p], W[2 * p + 1]], axis=1)  # [512, 128]
            for ch in range(4):
                blocks.append(Wp[P * ch:P * (ch + 1), :])
        return np.stack(blocks, 1).reshape(P, -1).astype(BF)

    wq_h, wk_h = stack_pairs(Wq), stack_pairs(Wk)
    # all-heads Wv, chunked by feature rows: col block ch = WvCat[128ch:128ch+128]
    WvCat = np.concatenate([Wv[h] for h in range(H)], axis=1)      # [512, 512]
    wv_h = WvCat.reshape(4, P, 512).transpose(1, 0, 2).reshape(P, -1).astype(BF)
    wo_h = np.stack([Wo[P * ch:P * (ch + 1), :] for ch in range(4)], 1)
    wo_h = wo_h.reshape(P, -1).astype(BF)

    def stack_bias(b, scale=1.0):
        cols = [np.concatenate([b[2 * p], b[2 * p + 1]]) * scale for p in range(NPAIR)]
        return np.stack(cols, 1).astype(np.float32)

    bq_h = stack_bias(bq, 0.125)
    bk_h = stack_bias(bk)
    # bv folded into the output bias: attn@(V + 1 bv^T) normalizes to +bv
    boP = (bo + np.concatenate([bv[h] for h in range(H)]) @ Wo).astype(BF)
    boB = np.ascontiguousarray(np.broadcast_to(boP, (P, D)))
    kl = np.arange(P)[:, None]
    ql = np.arange(P)[None, :]
    tril_strict = (kl > ql).astype(BF)

    def dram_chunks(m):
        # [128, n*512] SBUF image -> chunk-major [n, 128, 512] DRAM layout
        n = m.shape[1] // 512
        return np.ascontiguousarray(m.reshape(P, n, 512).transpose(1, 0, 2))

    def chunked_T(x, scale=1.0):
        # [S', D] -> [128, 4*S'] with col block ch = rows 128ch of x.T
        xT = np.ascontiguousarray(x.T) * scale
        return xT.reshape(4, P, -1).transpose(1, 0, 2).reshape(P, -1).astype(BF)

    def kmajor_T(x):
        # [S, D] -> [128, 16*512]: col 512*kt + 128*ch + c = x[128*kt+c, 128*ch+r]
        v4 = x.reshape(NKC, P, 4, P)            # (kt, c, ch, r)
        return np.ascontiguousarray(
            v4.transpose(3, 0, 2, 1)).reshape(P, -1).astype(BF)

    wq_h, wk_h, wv_h, wo_h = map(dram_chunks, (wq_h, wk_h, wv_h, wo_h))

    in_maps = []
    for c in range(8):
        b, pair = c // 2, c % 2
        sel = np.concatenate(
            [np.arange(P * (2 * i + pair), P * (2 * i + pair) + P) for i in range(NQT)])
        m = {
            "qT8": dram_chunks(chunked_T(query[b][sel], 0.125)),
            "kT": dram_chunks(chunked_T(key[b])),
            "vTk": dram_chunks(kmajor_T(value[b])),
            "wq": wq_h, "wk": wk_h, "wvN": wv_h, "wo": wo_h,
            "bq8": bq_h, "bk": bk_h, "boB": boB,
            "maskE": tril_strict if pair == 0 else np.zeros((P, P), BF),
            "maskO": np.ones((P, P), BF) if pair == 0 else tril_strict,
        }
        in_maps.append(m)
    return in_maps


def kernel(query, key, value, Wq, bq, Wk, bk, Wv, bv, Wo, bo):
    from concourse.bass_utils import run_bass_kernel_spmd

    args = [np.asarray(a, dtype=np.float32) for a in
            (query, key, value, Wq, bq, Wk, bk, Wv, bv, Wo, bo)]
    query, key, value, Wq, bq, Wk, bk, Wv, bv, Wo, bo = args

    nc = _build()
    in_maps = _host_prep(*args)
    res = run_bass_kernel_spmd(nc, in_maps, list(range(8)))

    out = np.empty((B, S, D), np.float32)
    for c in range(8):
        b, pair = c // 2, c % 2
        o = res.results[c]["out"]
        for i in range(NQT):
            g = 2 * i + pair
            out[b, P * g:P * (g + 1), :] = o[P * i:P * (i + 1), :]

    # q = S-1 attends to nothing -> reference softmax is uniform over all keys
    for b in range(B):
        vm = value[b].mean(0)
        x = np.concatenate([vm @ Wv[h] + bv[h] for h in range(H)])
        out[b, S - 1, :] = x @ Wo + bo
    return out


# revision 32
# speedup vs baseline: 1.1234x; 1.0067x over previous
"""Multi-head attention (strictly-upper-triangular mask variant) on 8 TRN2 cores.

Reference math (B=4, S=2048, D=512, H=8, A=64):
    q/k/v = per-head projections of query/key/value           [B,H,S,A]
    scores = q @ k^T / sqrt(A), masked where k <= q (lower triangle incl diag
    masked to -1e9 -> softmax attends strictly to FUTURE positions)
    out = concat_heads(softmax(scores) @ v) @ Wo + bo         [B,S,D]

Sharding: 8 cores = 4 batches x 2 interleaved q-tile sets.  Core c handles
batch b=c//2, q-tiles g = 2*i + (c%2) for i in 0..7 (128 rows each).  Every
core computes all 8 heads for its 1024 query rows; no collectives needed -
the host gather is a pure row-interleave concat.

v2 changes over the first working kernel (214us):
  * chunked input DMA ordered by first use, so the PE starts within ~2us
    instead of waiting ~24us for whole-tensor loads
  * V is projected directly into natural [k, a] layout (stationary = x^T
    key-tile, moving = all 8 heads' Wv) - eliminates 64 PE transposes and
    their PSUM/DVE traffic; the per-head [V | ones] stationary blocks for
    the AV matmul are assembled by one strided DVE copy per key tile
  * even/odd heads of a pair are interleaved per score matmul: the K=64
    stationaries auto-place on PE row-groups (0,0)/(64,0), so consecutive
    matmuls run concurrently in the array and their LDWEIGHTS overlap
  * score strips bin-packed into six [128,1536] fp32 PSUM groups per head
    (exact fit), exp'd in 6 ACT calls; PSUM budget 2x3 banks scores +
    2x1 bank shared (AV / projections / output) = 8
  * bv is folded into the output-projection bias on the host
    (attn@(V+1*bv^T) = attn@V + denom*bv -> normalized +bv -> bo' =
    bo + concat(bv)@Wo), so the V eviction is a plain copy
  * deep cross-pair pipeline: AV of pair p and projections of pair p+1
    fill the PE while pair p+1's exps run on ACT, keeping the PE HAM
    clock-gate warm

The single fully-masked query row (q = S-1, uniform attention in the
reference) comes back NaN from the device and is recomputed exactly on the
host during the gather.
"""

import numpy as np
import ml_dtypes

B, S, D, H, A = 4, 2048, 512, 8, 64
P = 128
NQ = 1024          # q rows per core
NQT = 8            # q tiles per core
NKC = 16           # k chunks
NPAIR = 4          # head pairs
BF = ml_dtypes.bfloat16

WKC = [P * (kc // 2 + 1) for kc in range(NKC)]   # score-strip width per k chunk
COFF = [0]
for _w in WKC:
    COFF.append(COFF[-1] + _w)                   # kc-ordered strip offsets, total 9216
WIN = 768                                        # score window width per head
NWIN = COFF[-1] // WIN                           # 12 windows
GW = 2 * WIN                                     # psum tile: [E window | O window]


def _pieces(c0, c1, splits):
    """Cut global column range [c0,c1) at window boundaries and at the
    window-local offsets in `splits`; yield (window, l0, l1)."""
    a = c0
    while a < c1:
        w = a // WIN
        nxt = WIN * (w + 1)
        for s in splits:
            g = WIN * w + s
            if a < g < nxt:
                nxt = g
        b = min(c1, nxt)
        yield (w, a - WIN * w, b - WIN * w)
        a = b

_cache = {}


def _split512(a, b):
    """Split [a,b) at multiples of 512 (PSUM bank boundaries)."""
    out = []
    while a < b:
        nxt = min(b, (a // 512 + 1) * 512)
        out.append((a, nxt))
        a = nxt
    return out


def _build():
    if "nc" in _cache:
        return _cache["nc"]

    import concourse.bacc as bacc
    import concourse.mybir as mybir
    import concourse.tile as tile

    F32 = mybir.dt.float32
    BF16 = mybir.dt.bfloat16
    MULT = mybir.AluOpType.mult
    EXP = mybir.ActivationFunctionType.Exp

    nc = bacc.Bacc("TRN2", target_bir_lowering=False, debug=False, num_devices=8)

    # big inputs are chunk-major [n, P, w] so every DMA reads DRAM sequentially
    qT8_d = nc.dram_tensor("qT8", [8, P, 512], BF16, kind="ExternalInput")
    kT_d = nc.dram_tensor("kT", [16, P, 512], BF16, kind="ExternalInput")
    vTk_d = nc.dram_tensor("vTk", [16, P, 512], BF16, kind="ExternalInput")
    wq_d = nc.dram_tensor("wq", [4, P, 512], BF16, kind="ExternalInput")
    wk_d = nc.dram_tensor("wk", [4, P, 512], BF16, kind="ExternalInput")
    wv_d = nc.dram_tensor("wvN", [4, P, 512], BF16, kind="ExternalInput")
    wo_d = nc.dram_tensor("wo", [4, P, 512], BF16, kind="ExternalInput")
    bq_d = nc.dram_tensor("bq8", [P, 4], F32, kind="ExternalInput")
    bk_d = nc.dram_tensor("bk", [P, 4], F32, kind="ExternalInput")
    bo_d = nc.dram_tensor("boB", [P, D], BF16, kind="ExternalInput")
    me_d = nc.dram_tensor("maskE", [P, P], BF16, kind="ExternalInput")
    mo_d = nc.dram_tensor("maskO", [P, P], BF16, kind="ExternalInput")
    out_d = nc.dram_tensor("out", [NQ, D], F32, kind="ExternalOutput")

    with tile.TileContext(nc) as tc:
        with (
            tc.tile_pool(name="cst", bufs=1) as cst,
            tc.tile_pool(name="act", bufs=1) as act,
            tc.tile_pool(name="ptg", bufs=16) as ptg,
            tc.tile_pool(name="rcp", bufs=3) as rcp,
            tc.tile_pool(name="ost", bufs=2) as ost,
            tc.tile_pool(name="sc", bufs=2, space="PSUM") as sc,
            tc.tile_pool(name="sm", bufs=2, space="PSUM") as sm,
        ):
            qT8 = cst.tile([P, 4 * NQ], BF16, tag="qT8")
            kT = cst.tile([P, 4 * S], BF16, tag="kT")
            vTk = cst.tile([P, NKC * 512], BF16, tag="vTk")
            wq = cst.tile([P, 2048], BF16, tag="wq")
            wk = cst.tile([P, 2048], BF16, tag="wk")
            wv = cst.tile([P, 2048], BF16, tag="wv")
            wo = cst.tile([P, 2048], BF16, tag="wo")
            bq = cst.tile([P, 4], F32, tag="bq")
            bk = cst.tile([P, 4], F32, tag="bk")
            boB = cst.tile([P, D], BF16, tag="boB")
            mE = cst.tile([P, P], BF16, tag="mE")
            mO = cst.tile([P, P], BF16, tag="mO")

            # small tensors first, then big ones chunked in first-use order
            for t, d in [(bq, bq_d), (bk, bk_d), (boB, bo_d),
                         (mE, me_d), (mO, mo_d)]:
                nc.sync.dma_start(t[:], d[:])

            def dma_chunks(t, d, order):
                for c in order:
                    nc.sync.dma_start(t[:, 512 * c:512 * (c + 1)], d[c, :, :])

            # chunk order = first-use order: Q block 0, K block 0, then the
            # rest; vTk follows kT so vdirect can run during pair-0 exps
            dma_chunks(wq, wq_d, range(4))
            dma_chunks(qT8, qT8_d, [0, 2, 4, 6])
            dma_chunks(wk, wk_d, range(4))
            dma_chunks(kT, kT_d, [0, 4, 8, 12])
            dma_chunks(qT8, qT8_d, [1, 3, 5, 7])
            dma_chunks(kT, kT_d, [4 * c + s for s in (1, 2, 3) for c in range(4)])
            dma_chunks(wv, wv_d, range(4))
            dma_chunks(vTk, vTk_d, range(16))
            dma_chunks(wo, wo_d, range(4))

            QT = [act.tile([P, NQ], BF16, tag=f"QT{p}", name=f"QT{p}") for p in range(NPAIR)]
            KT = [act.tile([P, S], BF16, tag=f"KT{p}", name=f"KT{p}") for p in range(NPAIR)]
            XT = [act.tile([P, NQ], BF16, tag=f"XT{c}", name=f"XT{c}") for c in range(4)]
            # per k-chunk: [V_h0 | ones | V_h1 | ones | ... | V_h7 | ones]
            Vk = act.tile([P, NKC * 1024], BF16, tag="Vk", name="Vk")

            nc.gpsimd.memset(
                Vk[:].rearrange("p (x t c) -> p x t c", t=2, c=64)[:, :, 1, :], 1.0)

            # ---- V directly in natural [k, a] layout for all 8 heads ----
            def vdirect(kts):
                for kt in kts:
                    psV = sm.tile([P, 512], F32, tag="sm")
                    for ch in range(4):
                        nc.tensor.matmul(
                            psV[:], vTk[:, 512 * kt + P * ch: 512 * kt + P * (ch + 1)],
                            wv[:, 512 * ch:512 * (ch + 1)],
                            start=(ch == 0), stop=(ch == 3))
                    dst = Vk[:].rearrange("p (k h t c) -> p k h t c", k=NKC, h=8, c=64)
                    nc.vector.tensor_copy(
                        dst[:, kt, :, 0, :],
                        psV[:].rearrange("p (h c) -> p h c", c=64))

            # ---- Q/K projections for one head pair ----
            def projQK(p):
                for qh in range(NQ // 512):
                    ps = sm.tile([P, 512], F32, tag="sm")
                    for ch in range(4):
                        nc.tensor.matmul(
                            ps[:], wq[:, (4 * p + ch) * P:(4 * p + ch + 1) * P],
                            qT8[:, NQ * ch + 512 * qh: NQ * ch + 512 * (qh + 1)],
                            start=(ch == 0), stop=(ch == 3))
                    nc.vector.tensor_scalar_add(
                        QT[p][:, 512 * qh:512 * (qh + 1)], ps[:], bq[:, p:p + 1])
                for sh in range(S // 512):
                    ps = sm.tile([P, 512], F32, tag="sm")
                    for ch in range(4):
                        nc.tensor.matmul(
                            ps[:], wk[:, (4 * p + ch) * P:(4 * p + ch + 1) * P],
                            kT[:, S * ch + 512 * sh: S * ch + 512 * (sh + 1)],
                            start=(ch == 0), stop=(ch == 3))
                    nc.vector.tensor_scalar_add(
                        KT[p][:, 512 * sh:512 * (sh + 1)], ps[:], bk[:, p:p + 1])

            # ---- scores + exp + mask for one head pair ----
            # One [128, 2*WIN] psum tile per 768-wide score window holds BOTH
            # heads (E cols 0:768, O cols 768:1536): the even/odd matmuls of a
            # strip become co-ready and run concurrently on PE row-groups
            # (0,0)/(64,0), and one exp covers both heads.  Matmul pieces are
            # cut at window boundaries and at psum bank boundaries (tile-local
            # 512 for E, 256 for O).
            def scores_pair(p):
                ptw = []
                for w in range(NWIN):
                    sw = sc.tile([P, GW], F32, tag="sc")
                    for kc in range(NKC):
                        c0, c1 = COFF[kc], COFF[kc + 1]
                        w0, w1 = max(c0, WIN * w), min(c1, WIN * (w + 1))
                        if w0 >= w1:
                            continue
                        pcs = [(0, list(_pieces(w0, w1, (512,)))),
                               (1, list(_pieces(w0, w1, (256,))))]
                        for i in range(max(len(pcs[0][1]), len(pcs[1][1]))):
                            for hh, pl in pcs:
                                if i >= len(pl):
                                    continue
                                _, l0, l1 = pl[i]
                                nc.tensor.matmul(
                                    sw[:, WIN * hh + l0: WIN * hh + l1],
                                    KT[p][64 * hh:64 * hh + 64, P * kc:P * (kc + 1)],
                                    QT[p][64 * hh:64 * hh + 64,
                                          WIN * w + l0 - c0: WIN * w + l1 - c0],
                                    start=True, stop=True)
                    pt = ptg.tile([P, GW], BF16, tag="pt")
                    nc.scalar.activation(pt[:], sw[:], EXP)
                    # diagonal masks whose last 128 columns land in this window
                    for kc in range(NKC):
                        d0 = COFF[kc + 1] - P
                        if not (WIN * w <= d0 < WIN * (w + 1)):
                            continue
                        ld = d0 - WIN * w
                        m = mE[:] if kc % 2 == 0 else mO[:]
                        for hh in (0, 1):
                            eng = nc.vector if (kc + hh) % 2 == 0 else nc.gpsimd
                            eng.tensor_tensor(
                                pt[:, WIN * hh + ld: WIN * hh + ld + P],
                                pt[:, WIN * hh + ld: WIN * hh + ld + P],
                                m, MULT)
                    ptw.append(pt)
                return ptw

            # AV accumulation per head: one psum bank per 512 q columns;
            # start=True only on the bank's first matmul (clears has_written
            # for the whole bank), later start=False matmuls
            # overwrite-where-unset / accumulate-where-set.  Issued kc-major
            # AFTER the pair's exps so each bank is held only briefly; pieces
            # are cut at score-window boundaries.
            def av_pair(p, ptw):
                for hh in (0, 1):
                    h = 2 * p + hh
                    hr = slice(64 * hh, 64 * hh + 64)
                    for b in range(2):
                        avb = sm.tile([P, 512], F32, tag="sm")
                        work = []
                        for kc in range(8 * b, NKC):
                            c0 = COFF[kc] + 512 * b
                            c1 = COFF[kc] + min(WKC[kc], 512 * (b + 1))
                            for (w, l0, l1) in _pieces(c0, c1, ()):
                                work.append((kc, w, l0, l1, c0))
                        for i, (kc, w, l0, l1, c0) in enumerate(work):
                            o0 = WIN * w + l0 - c0
                            nc.tensor.matmul(
                                avb[:, o0:o0 + (l1 - l0)],
                                Vk[:, 1024 * kc + P * h: 1024 * kc + P * (h + 1)],
                                ptw[w][:, WIN * hh + l0: WIN * hh + l1],
                                start=(i == 0), stop=(i == len(work) - 1),
                                skip_group_check=True)
                        # reciprocal is a custom DVE op that cannot read PSUM;
                        # bounce the replicated denominators through SBUF
                        r = rcp.tile([64, 1024], F32, tag="rec")
                        nc.vector.tensor_copy(r[:, 0:512], avb[64:128, :])
                        nc.vector.reciprocal_approx_fast(r[:, 512:1024], r[:, 0:512])
                        nc.vector.tensor_tensor(
                            XT[p][hr, 512 * b:512 * (b + 1)],
                            avb[0:64, :], r[:, 512:1024], MULT)

            # issue order = scheduler priority.  Per steady-state pair p:
            # scores(p) outrank the PE fillers (previous pair's AV, next
            # pair's projections) so the ACT exp stream never starves, and
            # the fillers outrank later pairs' psum-slot requests so slot
            # reservations don't serialize the pipeline.
            projQK(0)
            for p in range(NPAIR):
                pts = scores_pair(p)
                if p == 0:
                    vdirect(range(0, 8))
                if p + 1 < NPAIR:
                    projQK(p + 1)
                if p == 0:
                    vdirect(range(8, NKC))
                av_pair(p, pts)

            # ---- output projection ----
            for i in range(NQT):
                po = sm.tile([P, D], F32, tag="sm")
                for ch in range(4):
                    nc.tensor.matmul(po[:], XT[ch][:, P * i:P * (i + 1)],
                                     wo[:, 512 * ch:512 * (ch + 1)],
                                     start=(ch == 0), stop=(ch == 3))
                ob = ost.tile([P, D], F32, tag="ob")
                nc.vector.tensor_tensor(ob[:], po[:], boB[:],
                                        mybir.AluOpType.add)
                nc.sync.dma_start(out_d[P * i:P * (i + 1), :], ob[:])

    nc.compile()
    _cache["nc"] = nc
    return nc


def _host_prep(query, key, value, Wq, bq, Wk, bk, Wv, bv, Wo, bo):
    """Build the 8 per-core input maps (all device-side layouts)."""
    def stack_pairs(W):
        # [H,D,A] -> [128, 16*128]: col block (4p+ch) = rows 128ch of [W_2p|W_2p+1]
        blocks = []
        for p in range(NPAIR):
            Wp = np.concatenate([W[2 * p], W[2 * p + 1]], axis=1)  # [512, 128]
            for ch in range(4):
                blocks.append(Wp[P * ch:P * (ch + 1), :])
        return np.stack(blocks, 1).reshape(P, -1).astype(BF)

    wq_h, wk_h = stack_pairs(Wq), stack_pairs(Wk)
    # all-heads Wv, chunked by feature rows: col block ch = WvCat[128ch:128ch+128]
    WvCat = np.concatenate([Wv[h] for h in range(H)], axis=1)      # [512, 512]
    wv_h = WvCat.reshape(4, P, 512).transpose(1, 0, 2).reshape(P, -1).astype(BF)
    wo_h = np.stack([Wo[P * ch:P * (ch + 1), :] for ch in range(4)], 1)
    wo_h = wo_h.reshape(P, -1).astype(BF)

    def stack_bias(b, scale=1.0):
        cols = [np.concatenate([b[2 * p], b[2 * p + 1]]) * scale for p in range(NPAIR)]
        return np.stack(cols, 1).astype(np.float32)

    bq_h = stack_bias(bq, 0.125)
    bk_h = stack_bias(bk)
    # bv folded into the output bias: attn@(V + 1 bv^T) normalizes to +bv
    boP = (bo + np.concatenate([bv[h] for h in range(H)]) @ Wo).astype(BF)
    boB = np.ascontiguousarray(np.broadcast_to(boP, (P, D)))
    kl = np.arange(P)[:, None]
    ql = np.arange(P)[None, :]
    tril_strict = (kl > ql).astype(BF)

    def dram_chunks(m):
        # [128, n*512] SBUF image -> chunk-major [n, 128, 512] DRAM layout
        n = m.shape[1] // 512
        return np.ascontiguousarray(m.reshape(P, n, 512).transpose(1, 0, 2))

    def chunked_T(x, scale=1.0):
        # [S', D] -> [128, 4*S'] with col block ch = rows 128ch of x.T
        xT = np.ascontiguousarray(x.T) * scale
        return xT.reshape(4, P, -1).transpose(1, 0, 2).reshape(P, -1).astype(BF)

    def kmajor_T(x):
        # [S, D] -> [128, 16*512]: col 512*kt + 128*ch + c = x[128*kt+c, 128*ch+r]
        v4 = x.reshape(NKC, P, 4, P)            # (kt, c, ch, r)
        return np.ascontiguousarray(
            v4.transpose(3, 0, 2, 1)).reshape(P, -1).astype(BF)

    wq_h, wk_h, wv_h, wo_h = map(dram_chunks, (wq_h, wk_h, wv_h, wo_h))

    in_maps = []
    for c in range(8):
        b, pair = c // 2, c % 2
        sel = np.concatenate(
            [np.arange(P * (2 * i + pair), P * (2 * i + pair) + P) for i in range(NQT)])
        m = {
            "qT8": dram_chunks(chunked_T(query[b][sel], 0.125)),
            "kT": dram_chunks(chunked_T(key[b])),
            "vTk": dram_chunks(kmajor_T(value[b])),
            "wq": wq_h, "wk": wk_h, "wvN": wv_h, "wo": wo_h,
            "bq8": bq_h, "bk": bk_h, "boB": boB,
            "maskE": tril_strict if pair == 0 else np.zeros((P, P), BF),
            "maskO": np.ones((P, P), BF) if pair == 0 else tril_strict,
        }
        in_maps.append(m)
    return in_maps


def kernel(query, key, value, Wq, bq, Wk, bk, Wv, bv, Wo, bo):
    from concourse.bass_utils import run_bass_kernel_spmd

    args = [np.asarray(a, dtype=np.float32) for a in
            (query, key, value, Wq, bq, Wk, bk, Wv, bv, Wo, bo)]
    query, key, value, Wq, bq, Wk, bk, Wv, bv, Wo, bo = args

    nc = _build()
    in_maps = _host_prep(*args)
    res = run_bass_kernel_spmd(nc, in_maps, list(range(8)))

    out = np.empty((B, S, D), np.float32)
    for c in range(8):
        b, pair = c // 2, c % 2
        o = res.results[c]["out"]
        for i in range(NQT):
            g = 2 * i + pair
            out[b, P * g:P * (g + 1), :] = o[P * i:P * (i + 1), :]

    # q = S-1 attends to nothing -> reference softmax is uniform over all keys
    for b in range(B):
        vm = value[b].mean(0)
        x = np.concatenate([vm @ Wv[h] + bv[h] for h in range(H)])
        out[b, S - 1, :] = x @ Wo + bo
    return out


# revision 34
# speedup vs baseline: 1.1584x; 1.0311x over previous
"""Multi-head attention (strictly-upper-triangular mask variant) on 8 TRN2 cores.

Reference math (B=4, S=2048, D=512, H=8, A=64):
    q/k/v = per-head projections of query/key/value           [B,H,S,A]
    scores = q @ k^T / sqrt(A), masked where k <= q (lower triangle incl diag
    masked to -1e9 -> softmax attends strictly to FUTURE positions)
    out = concat_heads(softmax(scores) @ v) @ Wo + bo         [B,S,D]

Sharding: 8 cores = 4 batches x 2 interleaved q-tile sets.  Core c handles
batch b=c//2, q-tiles g = 2*i + (c%2) for i in 0..7 (128 rows each).  Every
core computes all 8 heads for its 1024 query rows; no collectives needed -
the host gather is a pure row-interleave concat.

Optimizations over the first working kernel (214us -> 158us):
  * chunk-contiguous DRAM layouts ([n,128,512] blocks) + DMAs ordered by
    first use, so projections start ~15us in instead of ~32us
  * V is projected directly into natural [k, a] layout (stationary = x^T
    key-tile, moving = all 8 heads' Wv) - eliminates 64 PE transposes and
    their PSUM/DVE traffic; the per-head [V | ones] stationary blocks for
    the AV matmul are assembled by one strided DVE copy per key tile
  * score columns (kc-ordered strips, 9216 per head) are processed in
    twelve 768-wide windows; ONE [128,1536] fp32 psum tile per window
    holds BOTH heads of the pair side by side, so the even/odd K=64 score
    matmuls are co-ready, land on PE row-groups (0,0)/(64,0), and run
    CONCURRENTLY in the array (second matmul of a pair costs ~3ns); one
    exp per window covers both heads (12 ACT calls/pair)
  * PSUM budget: 2 window tiles (3 banks each) + 2 shared [128,512] banks
    (AV accumulation / projections / V-direct / output projection) = 8
  * bv is folded into the output-projection bias on the host
    (attn@(V+1*bv^T) = attn@V + denom*bv -> normalized +bv -> bo' =
    bo + concat(bv)@Wo), so the V eviction is a plain copy; bo' enters
    via a broadcast-bias DVE eviction (no bias matmul)
  * diagonal-block masks split across DVE and GpSimd engines
  * issue order tuned against the Tile scheduler's priority + psum-slot
    reservation semantics: scores(p) outrank the PE fillers (vdirect,
    projQK(p+1), av(p)) so the ACT exp stream never starves; AV matmuls
    iterate kc-ascending so they overlap their own pair's exps

The single fully-masked query row (q = S-1, uniform attention in the
reference) comes back NaN from the device and is recomputed exactly on the
host during the gather.
"""

import numpy as np
import ml_dtypes

B, S, D, H, A = 4, 2048, 512, 8, 64
P = 128
NQ = 1024          # q rows per core
NQT = 8            # q tiles per core
NKC = 16           # k chunks
NPAIR = 4          # head pairs
BF = ml_dtypes.bfloat16

WKC = [P * (kc // 2 + 1) for kc in range(NKC)]   # score-strip width per k chunk
COFF = [0]
for _w in WKC:
    COFF.append(COFF[-1] + _w)                   # kc-ordered strip offsets, total 9216
WIN = 768                                        # score window width per head
NWIN = COFF[-1] // WIN                           # 12 windows
GW = 2 * WIN                                     # psum tile: [E window | O window]


def _pieces(c0, c1, splits):
    """Cut global column range [c0,c1) at window boundaries and at the
    window-local offsets in `splits`; yield (window, l0, l1)."""
    a = c0
    while a < c1:
        w = a // WIN
        nxt = WIN * (w + 1)
        for s in splits:
            g = WIN * w + s
            if a < g < nxt:
                nxt = g
        b = min(c1, nxt)
        yield (w, a - WIN * w, b - WIN * w)
        a = b

_cache = {}


def _split512(a, b):
    """Split [a,b) at multiples of 512 (PSUM bank boundaries)."""
    out = []
    while a < b:
        nxt = min(b, (a // 512 + 1) * 512)
        out.append((a, nxt))
        a = nxt
    return out


def _build():
    if "nc" in _cache:
        return _cache["nc"]

    import concourse.bacc as bacc
    import concourse.mybir as mybir
    import concourse.tile as tile

    F32 = mybir.dt.float32
    BF16 = mybir.dt.bfloat16
    MULT = mybir.AluOpType.mult
    EXP = mybir.ActivationFunctionType.Exp

    nc = bacc.Bacc("TRN2", target_bir_lowering=False, debug=False, num_devices=8)

    # big inputs are chunk-major [n, P, w] so every DMA reads DRAM sequentially
    qT8_d = nc.dram_tensor("qT8", [8, P, 512], BF16, kind="ExternalInput")
    kT_d = nc.dram_tensor("kT", [16, P, 512], BF16, kind="ExternalInput")
    vTk_d = nc.dram_tensor("vTk", [16, P, 512], BF16, kind="ExternalInput")
    wq_d = nc.dram_tensor("wq", [4, P, 512], BF16, kind="ExternalInput")
    wk_d = nc.dram_tensor("wk", [4, P, 512], BF16, kind="ExternalInput")
    wv_d = nc.dram_tensor("wvN", [4, P, 512], BF16, kind="ExternalInput")
    wo_d = nc.dram_tensor("wo", [4, P, 512], BF16, kind="ExternalInput")
    bq_d = nc.dram_tensor("bq8", [P, 4], F32, kind="ExternalInput")
    bk_d = nc.dram_tensor("bk", [P, 4], F32, kind="ExternalInput")
    bo_d = nc.dram_tensor("boB", [P, D], BF16, kind="ExternalInput")
    me_d = nc.dram_tensor("maskE", [P, P], BF16, kind="ExternalInput")
    mo_d = nc.dram_tensor("maskO", [P, P], BF16, kind="ExternalInput")
    out_d = nc.dram_tensor("out", [NQ, D], F32, kind="ExternalOutput")

    with tile.TileContext(nc) as tc:
        with (
            tc.tile_pool(name="cst", bufs=1) as cst,
            tc.tile_pool(name="act", bufs=1) as act,
            tc.tile_pool(name="ptg", bufs=16) as ptg,
            tc.tile_pool(name="rcp", bufs=3) as rcp,
            tc.tile_pool(name="ost", bufs=2) as ost,
            tc.tile_pool(name="sc", bufs=2, space="PSUM") as sc,
            tc.tile_pool(name="sm", bufs=2, space="PSUM") as sm,
        ):
            qT8 = cst.tile([P, 4 * NQ], BF16, tag="qT8")
            kT = cst.tile([P, 4 * S], BF16, tag="kT")
            vTk = cst.tile([P, NKC * 512], BF16, tag="vTk")
            wq = cst.tile([P, 2048], BF16, tag="wq")
            wk = cst.tile([P, 2048], BF16, tag="wk")
            wv = cst.tile([P, 2048], BF16, tag="wv")
            wo = cst.tile([P, 2048], BF16, tag="wo")
            bq = cst.tile([P, 4], F32, tag="bq")
            bk = cst.tile([P, 4], F32, tag="bk")
            boB = cst.tile([P, D], BF16, tag="boB")
            mE = cst.tile([P, P], BF16, tag="mE")
            mO = cst.tile([P, P], BF16, tag="mO")

            # small tensors first, then big ones chunked in first-use order
            for t, d in [(bq, bq_d), (bk, bk_d), (boB, bo_d),
                         (mE, me_d), (mO, mo_d)]:
                nc.sync.dma_start(t[:], d[:])

            def dma_chunks(t, d, order):
                for c in order:
                    nc.sync.dma_start(t[:, 512 * c:512 * (c + 1)], d[c, :, :])

            # chunk order = first-use order of the projection block loops
            dma_chunks(wq, wq_d, range(4))
            dma_chunks(qT8, qT8_d, [0, 2, 4, 6, 1, 3, 5, 7])
            dma_chunks(wk, wk_d, range(4))
            dma_chunks(kT, kT_d, [4 * c + s for s in range(4) for c in range(4)])
            dma_chunks(wv, wv_d, range(4))
            dma_chunks(vTk, vTk_d, range(16))
            dma_chunks(wo, wo_d, range(4))

            QT = [act.tile([P, NQ], BF16, tag=f"QT{p}", name=f"QT{p}") for p in range(NPAIR)]
            KT = [act.tile([P, S], BF16, tag=f"KT{p}", name=f"KT{p}") for p in range(NPAIR)]
            XT = [act.tile([P, NQ], BF16, tag=f"XT{c}", name=f"XT{c}") for c in range(4)]
            # per k-chunk: [V_h0 | ones | V_h1 | ones | ... | V_h7 | ones]
            Vk = act.tile([P, NKC * 1024], BF16, tag="Vk", name="Vk")

            nc.gpsimd.memset(
                Vk[:].rearrange("p (x t c) -> p x t c", t=2, c=64)[:, :, 1, :], 1.0)

            # ---- V directly in natural [k, a] layout for all 8 heads ----
            def vdirect():
                for kt in range(NKC):
                    psV = sm.tile([P, 512], F32, tag="sm")
                    for ch in range(4):
                        nc.tensor.matmul(
                            psV[:], vTk[:, 512 * kt + P * ch: 512 * kt + P * (ch + 1)],
                            wv[:, 512 * ch:512 * (ch + 1)],
                            start=(ch == 0), stop=(ch == 3))
                    dst = Vk[:].rearrange("p (k h t c) -> p k h t c", k=NKC, h=8, c=64)
                    nc.vector.tensor_copy(
                        dst[:, kt, :, 0, :],
                        psV[:].rearrange("p (h c) -> p h c", c=64))

            # ---- Q/K projections for one head pair ----
            def projQK(p):
                for qh in range(NQ // 512):
                    ps = sm.tile([P, 512], F32, tag="sm")
                    for ch in range(4):
                        nc.tensor.matmul(
                            ps[:], wq[:, (4 * p + ch) * P:(4 * p + ch + 1) * P],
                            qT8[:, NQ * ch + 512 * qh: NQ * ch + 512 * (qh + 1)],
                            start=(ch == 0), stop=(ch == 3))
                    nc.vector.tensor_scalar_add(
                        QT[p][:, 512 * qh:512 * (qh + 1)], ps[:], bq[:, p:p + 1])
                for sh in range(S // 512):
                    ps = sm.tile([P, 512], F32, tag="sm")
                    for ch in range(4):
                        nc.tensor.matmul(
                            ps[:], wk[:, (4 * p + ch) * P:(4 * p + ch + 1) * P],
                            kT[:, S * ch + 512 * sh: S * ch + 512 * (sh + 1)],
                            start=(ch == 0), stop=(ch == 3))
                    nc.vector.tensor_scalar_add(
                        KT[p][:, 512 * sh:512 * (sh + 1)], ps[:], bk[:, p:p + 1])

            # ---- scores + exp + mask for one head pair ----
            # One [128, 2*WIN] psum tile per 768-wide score window holds BOTH
            # heads (E cols 0:768, O cols 768:1536): the even/odd matmuls of a
            # strip become co-ready and run concurrently on PE row-groups
            # (0,0)/(64,0), and one exp covers both heads.  Matmul pieces are
            # cut at window boundaries and at psum bank boundaries (tile-local
            # 512 for E, 256 for O).
            def scores_pair(p):
                ptw = []
                for w in range(NWIN):
                    sw = sc.tile([P, GW], F32, tag="sc")
                    for kc in range(NKC):
                        c0, c1 = COFF[kc], COFF[kc + 1]
                        w0, w1 = max(c0, WIN * w), min(c1, WIN * (w + 1))
                        if w0 >= w1:
                            continue
                        pcs = [(0, list(_pieces(w0, w1, (512,)))),
                               (1, list(_pieces(w0, w1, (256,))))]
                        for i in range(max(len(pcs[0][1]), len(pcs[1][1]))):
                            for hh, pl in pcs:
                                if i >= len(pl):
                                    continue
                                _, l0, l1 = pl[i]
                                nc.tensor.matmul(
                                    sw[:, WIN * hh + l0: WIN * hh + l1],
                                    KT[p][64 * hh:64 * hh + 64, P * kc:P * (kc + 1)],
                                    QT[p][64 * hh:64 * hh + 64,
                                          WIN * w + l0 - c0: WIN * w + l1 - c0],
                                    start=True, stop=True)
                    pt = ptg.tile([P, GW], BF16, tag="pt")
                    nc.scalar.activation(pt[:], sw[:], EXP)
                    # diagonal masks whose last 128 columns land in this window
                    for kc in range(NKC):
                        d0 = COFF[kc + 1] - P
                        if not (WIN * w <= d0 < WIN * (w + 1)):
                            continue
                        ld = d0 - WIN * w
                        m = mE[:] if kc % 2 == 0 else mO[:]
                        for hh in (0, 1):
                            eng = nc.vector if (kc + hh) % 2 == 0 else nc.gpsimd
                            eng.tensor_tensor(
                                pt[:, WIN * hh + ld: WIN * hh + ld + P],
                                pt[:, WIN * hh + ld: WIN * hh + ld + P],
                                m, MULT)
                    ptw.append(pt)
                return ptw

            # AV accumulation per head: one psum bank per 512 q columns;
            # start=True only on the bank's first matmul (clears has_written
            # for the whole bank), later start=False matmuls
            # overwrite-where-unset / accumulate-where-set.  Issued kc-major
            # AFTER the pair's exps so each bank is held only briefly; pieces
            # are cut at score-window boundaries.
            def av_pair(p, ptw):
                for hh in (0, 1):
                    h = 2 * p + hh
                    hr = slice(64 * hh, 64 * hh + 64)
                    for b in range(2):
                        avb = sm.tile([P, 512], F32, tag="sm")
                        work = []
                        for kc in range(8 * b, NKC):
                            c0 = COFF[kc] + 512 * b
                            c1 = COFF[kc] + min(WKC[kc], 512 * (b + 1))
                            for (w, l0, l1) in _pieces(c0, c1, ()):
                                work.append((kc, w, l0, l1, c0))
                        for i, (kc, w, l0, l1, c0) in enumerate(work):
                            o0 = WIN * w + l0 - c0
                            nc.tensor.matmul(
                                avb[:, o0:o0 + (l1 - l0)],
                                Vk[:, 1024 * kc + P * h: 1024 * kc + P * (h + 1)],
                                ptw[w][:, WIN * hh + l0: WIN * hh + l1],
                                start=(i == 0), stop=(i == len(work) - 1),
                                skip_group_check=True)
                        # reciprocal is a custom DVE op that cannot read PSUM;
                        # bounce the replicated denominators through SBUF
                        r = rcp.tile([64, 1024], F32, tag="rec")
                        nc.vector.tensor_copy(r[:, 0:512], avb[64:128, :])
                        nc.vector.reciprocal_approx_fast(r[:, 512:1024], r[:, 0:512])
                        nc.vector.tensor_tensor(
                            XT[p][hr, 512 * b:512 * (b + 1)],
                            avb[0:64, :], r[:, 512:1024], MULT)

            # issue order = scheduler priority.  Per steady-state pair p:
            # scores(p) outrank the PE fillers (previous pair's AV, next
            # pair's projections) so the ACT exp stream never starves, and
            # the fillers outrank later pairs' psum-slot requests so slot
            # reservations don't serialize the pipeline.
            projQK(0)
            for p in range(NPAIR):
                pts = scores_pair(p)
                if p == 0:
                    vdirect()
                if p + 1 < NPAIR:
                    projQK(p + 1)
                av_pair(p, pts)

            # ---- output projection ----
            for i in range(NQT):
                po = sm.tile([P, D], F32, tag="sm")
                for ch in range(4):
                    nc.tensor.matmul(po[:], XT[ch][:, P * i:P * (i + 1)],
                                     wo[:, 512 * ch:512 * (ch + 1)],
                                     start=(ch == 0), stop=(ch == 3))
                ob = ost.tile([P, D], F32, tag="ob")
                nc.vector.tensor_tensor(ob[:], po[:], boB[:],
                                        mybir.AluOpType.add)
                nc.sync.dma_start(out_d[P * i:P * (i + 1), :], ob[:])

    nc.compile()
    _cache["nc"] = nc
    return nc


def _host_prep(query, key, value, Wq, bq, Wk, bk, Wv, bv, Wo, bo):
    """Build the 8 per-core input maps (all device-side layouts)."""
    def stack_pairs(W):
        # [H,D,A] -> [128, 16*128]: col block (4p+ch) = rows 128ch of [W_2p|W_2p+1]
        blocks = []
        for p in range(NPAIR):
            Wp = np.concatenate([W[2 * p], W[2 * p + 1]], axis=1)  # [512, 128]
            for ch in range(4):
                blocks.append(Wp[P * ch:P * (ch + 1), :])
        return np.stack(blocks, 1).reshape(P, -1).astype(BF)

    wq_h, wk_h = stack_pairs(Wq), stack_pairs(Wk)
    # all-heads Wv, chunked by feature rows: col block ch = WvCat[128ch:128ch+128]
    WvCat = np.concatenate([Wv[h] for h in range(H)], axis=1)      # [512, 512]
    wv_h = WvCat.reshape(4, P, 512).transpose(1, 0, 2).reshape(P, -1).astype(BF)
    wo_h = np.stack([Wo[P * ch:P * (ch + 1), :] for ch in range(4)], 1)
    wo_h = wo_h.reshape(P, -1).astype(BF)

    def stack_bias(b, scale=1.0):
        cols = [np.concatenate([b[2 * p], b[2 * p + 1]]) * scale for p in range(NPAIR)]
        return np.stack(cols, 1).astype(np.float32)

    bq_h = stack_bias(bq, 0.125)
    bk_h = stack_bias(bk)
    # bv folded into the output bias: attn@(V + 1 bv^T) normalizes to +bv
    boP = (bo + np.concatenate([bv[h] for h in range(H)]) @ Wo).astype(BF)
    boB = np.ascontiguousarray(np.broadcast_to(boP, (P, D)))
    kl = np.arange(P)[:, None]
    ql = np.arange(P)[None, :]
    tril_strict = (kl > ql).astype(BF)

    def dram_chunks(m):
        # [128, n*512] SBUF image -> chunk-major [n, 128, 512] DRAM layout
        n = m.shape[1] // 512
        return np.ascontiguousarray(m.reshape(P, n, 512).transpose(1, 0, 2))

    def chunked_T(x, scale=1.0):
        # [S', D] -> [128, 4*S'] with col block ch = rows 128ch of x.T
        xT = np.ascontiguousarray(x.T) * scale
        return xT.reshape(4, P, -1).transpose(1, 0, 2).reshape(P, -1).astype(BF)

    def kmajor_T(x):
        # [S, D] -> [128, 16*512]: col 512*kt + 128*ch + c = x[128*kt+c, 128*ch+r]
        v4 = x.reshape(NKC, P, 4, P)            # (kt, c, ch, r)
        return np.ascontiguousarray(
            v4.transpose(3, 0, 2, 1)).reshape(P, -1).astype(BF)

    wq_h, wk_h, wv_h, wo_h = map(dram_chunks, (wq_h, wk_h, wv_h, wo_h))

    in_maps = []
    for c in range(8):
        b, pair = c // 2, c % 2
        sel = np.concatenate(
            [np.arange(P * (2 * i + pair), P * (2 * i + pair) + P) for i in range(NQT)])
        m = {
            "qT8": dram_chunks(chunked_T(query[b][sel], 0.125)),
            "kT": dram_chunks(chunked_T(key[b])),
            "vTk": dram_chunks(kmajor_T(value[b])),
            "wq": wq_h, "wk": wk_h, "wvN": wv_h, "wo": wo_h,
            "bq8": bq_h, "bk": bk_h, "boB": boB,
            "maskE": tril_strict if pair == 0 else np.zeros((P, P), BF),
            "maskO": np.ones((P, P), BF) if pair == 0 else tril_strict,
        }
        in_maps.append(m)
    return in_maps


def kernel(query, key, value, Wq, bq, Wk, bk, Wv, bv, Wo, bo):
    from concourse.bass_utils import run_bass_kernel_spmd

    args = [np.asarray(a, dtype=np.float32) for a in
            (query, key, value, Wq, bq, Wk, bk, Wv, bv, Wo, bo)]
    query, key, value, Wq, bq, Wk, bk, Wv, bv, Wo, bo = args

    nc = _build()
    in_maps = _host_prep(*args)
    res = run_bass_kernel_spmd(nc, in_maps, list(range(8)))

    out = np.empty((B, S, D), np.float32)
    for c in range(8):
        b, pair = c // 2, c % 2
        o = res.results[c]["out"]
        for i in range(NQT):
            g = 2 * i + pair
            out[b, P * g:P * (g + 1), :] = o[P * i:P * (i + 1), :]

    # q = S-1 attends to nothing -> reference softmax is uniform over all keys
    for b in range(B):
        vm = value[b].mean(0)
        x = np.concatenate([vm @ Wv[h] + bv[h] for h in range(H)])
        out[b, S - 1, :] = x @ Wo + bo
    return out
